# revision 1
# baseline (speedup 1.0000x reference)
"""Trainium2 Bass kernel for nn_EncoderOnlyBlock (4-head full-dim encoder block).

Sharding: fully data-parallel, no collectives. 8 cores = (batch b, seq-half).
Each core computes its 1024 query tokens end-to-end for all 4 heads; K work
for the full 2048-token batch row is recomputed on both cores of a batch
(the only duplicated work).

Per-core math (all matmuls bf16 inputs, fp32 PSUM accumulation):
  x_perm = [own-half tokens; other-half tokens]           (host permute)
  Q^T_h = Wq_h^T @ x_perm^T[:, :1024] + bq_h              [e, si]
  K^T_h = Wk_h^T @ x_perm^T                               [e, sj]   (bk dropped:
          softmax rows are invariant to the q.bk and bq.bk terms)
  S     = Q^T^T K^T / sqrt(D); A = exp(S) / rowsum        (no max-sub: |S|<2)
  M_h   = x_perm^T @ A^T                                  [d, si]   (A@V == Wv^T@M
  hd^T_h= Wv_h^T @ M_h                                    [e, si]    reassociation,
          bv_h folded into cvec since rows of A sum to 1)
  proj  = sum_h hd_h @ W1_h + cvec,  cvec = b1 + sum_h bv_h @ W1_h  (host)
  u1    = x_res + proj;  yhat = (u1-mu1)*rsqrt(var1+eps);  y = yhat*g1+be1
  u2    = y + y@W2 + b2 = yhat@W2' + bu (+ y)  with W2' = g1*W2, bu = b2+be1@W2
  out   = (u2-mu2)*rsqrt(var2+eps) * g2 + be2
LN means/vars via sum & sum-of-squares accumulators (E[x^2]-mu^2); g1/be1 and
g2/be2 application is skipped when they are exactly ones/zeros (checked on host;
g1/be1 additionally fold into W2'/bu which is exact in that case).
"""

import numpy as np
import ml_dtypes

BF = ml_dtypes.bfloat16
P = 128
D = 1024
S = 2048
SI = 1024
H = 4
ET = D // P       # 8 e/d/f 128-blocks
SJT = S // P      # 16 sj 128-blocks
SIT = SI // P     # 8 si 128-blocks
SCALE = 1.0 / 32.0  # 1/sqrt(D)
EPS = 1e-5

_CACHE = {}


def _emit(nc, tc, A, trivial_gbe):
    """Emit the per-core program. A: dict name -> dram AP."""
    from contextlib import ExitStack

    import concourse.bass as bass
    import concourse.mybir as mybir
    from concourse.masks import make_identity

    f32 = mybir.dt.float32
    bf16 = mybir.dt.bfloat16
    Act = mybir.ActivationFunctionType
    Alu = mybir.AluOpType

    with ExitStack() as ctx:
        consts = ctx.enter_context(tc.tile_pool(name="consts", bufs=1))
        psA = ctx.enter_context(tc.tile_pool(name="psA", bufs=3, space="PSUM"))
        psB = ctx.enter_context(tc.tile_pool(name="psB", bufs=2, space="PSUM"))

        ident = consts.tile([P, P], bf16, tag="ident")
        make_identity(nc, ident[:])
        bqr_sb = consts.tile([P, H * ET], f32, tag="bqr")
        nc.sync.dma_start(out=bqr_sb[:], in_=A["bqr"][:])
        cvec_sb = consts.tile([1, D], bf16, tag="cvec")
        nc.sync.dma_start(out=cvec_sb[:], in_=A["cvec"][:])
        buv_sb = consts.tile([1, D], bf16, tag="buv")
        nc.sync.dma_start(out=buv_sb[:], in_=A["buv"][:])
        ones_sb = consts.tile([1, P], bf16, tag="ones")
        nc.vector.memset(ones_sb[:], 1.0)
        eps_sb = consts.tile([P, 1], f32, tag="eps")
        nc.vector.memset(eps_sb[:], EPS)

        head_ctx = ExitStack()
        xpool = head_ctx.enter_context(tc.tile_pool(name="xp", bufs=1))
        wqkv_pool = head_ctx.enter_context(tc.tile_pool(name="wqkv", bufs=3))
        w1_pool = head_ctx.enter_context(tc.tile_pool(name="w1", bufs=8))
        qt_pool = head_ctx.enter_context(tc.tile_pool(name="qt", bufs=1))
        kt_pool = head_ctx.enter_context(tc.tile_pool(name="kt", bufs=1))
        attn_pool = head_ctx.enter_context(tc.tile_pool(name="at", bufs=3))
        atT_pool = head_ctx.enter_context(tc.tile_pool(name="atT", bufs=1))
        m_pool = head_ctx.enter_context(tc.tile_pool(name="m", bufs=1))
        ht_pool = head_ctx.enter_context(tc.tile_pool(name="ht", bufs=1))
        proj_pool = head_ctx.enter_context(tc.tile_pool(name="pj", bufs=1))
        red_pool = head_ctx.enter_context(tc.tile_pool(name="red", bufs=8))

        # x^T tiles first (first K-chain consumes them), x natural after head-0's
        # weights are queued (not needed until the M phase of head 0)
        xt_sb = xpool.tile([P, ET, S], bf16, tag="xt")
        for c in range(ET):
            nc.sync.dma_start(out=xt_sb[:, c, :], in_=A["xt"][c * P:(c + 1) * P, :])
        xn_sb = xpool.tile([P, SJT, D], bf16, tag="xn")

        proj_sb = proj_pool.tile([P, SIT, D], bf16, tag="proj")

        for h in range(H):
            # ---- K^T = Wk^T @ x^T : [e, sj]
            kt_sb = kt_pool.tile([P, ET, S], bf16, tag="kt")
            for c in range(ET):
                wk_c = wqkv_pool.tile([P, ET, P], bf16, tag="wqkv")
                nc.sync.dma_start(out=wk_c[:], in_=A["wkb"][h, c])
                for hs in range(2):
                    ps = psA.tile([P, 1024], f32, tag="psA")
                    for nb in range(2):
                        for kc in range(ET):
                            nc.tensor.matmul(
                                ps[:, nb * 512:(nb + 1) * 512],
                                lhsT=wk_c[:, kc, :],
                                rhs=xt_sb[:, kc, hs * 1024 + nb * 512:hs * 1024 + (nb + 1) * 512],
                                start=(kc == 0), stop=(kc == ET - 1),
                            )
                    nc.vector.tensor_copy(kt_sb[:, c, hs * 1024:(hs + 1) * 1024], ps[:])

            # ---- Q^T = Wq^T @ x^T[:, :1024] + bq : [e, si]
            qt_sb = qt_pool.tile([P, ET, SI], bf16, tag="qt")
            for c in range(ET):
                wq_c = wqkv_pool.tile([P, ET, P], bf16, tag="wqkv")
                nc.sync.dma_start(out=wq_c[:], in_=A["wqb"][h, c])
                ps = psA.tile([P, 1024], f32, tag="psA")
                for nb in range(2):
                    for kc in range(ET):
                        nc.tensor.matmul(
                            ps[:, nb * 512:(nb + 1) * 512],
                            lhsT=wq_c[:, kc, :],
                            rhs=xt_sb[:, kc, nb * 512:(nb + 1) * 512],
                            start=(kc == 0), stop=(kc == ET - 1),
                        )
                nc.scalar.activation(
                    out=qt_sb[:, c, :], in_=ps[:], func=Act.Identity,
                    bias=bqr_sb[:, h * ET + c:h * ET + c + 1],
                )

            if h == 0:
                for j in range(SJT):
                    nc.sync.dma_start(out=xn_sb[:, j, :], in_=A["xn"][j * P:(j + 1) * P, :])

            # ---- attention: scores+softmax per si-tile, transposes pipelined
            # two tiles behind so the last softmax hides under the next scores
            m_sb = m_pool.tile([P, ET, SI], bf16, tag="m")
            attn_tiles = [None] * SIT
            at_tiles = [None] * 4

            def scores_softmax(t):
                a_t = attn_pool.tile([P, S], bf16, tag="attn")
                attn_tiles[t] = a_t
                r = red_pool.tile([P, 2], f32, tag="rsum")
                rec = red_pool.tile([P, 1], f32, tag="rec")
                for hs in range(2):
                    ps = psA.tile([P, 1024], f32, tag="psA")
                    for nb in range(2):
                        for kc in range(ET):
                            nc.tensor.matmul(
                                ps[:, nb * 512:(nb + 1) * 512],
                                lhsT=qt_sb[:, kc, t * P:(t + 1) * P],
                                rhs=kt_sb[:, kc, hs * 1024 + nb * 512:hs * 1024 + (nb + 1) * 512],
                                start=(kc == 0), stop=(kc == ET - 1),
                            )
                    nc.scalar.activation(
                        out=a_t[:, hs * 1024:(hs + 1) * 1024], in_=ps[:],
                        func=Act.Exp, scale=SCALE,
                        accum_out=r[:, hs:hs + 1],
                    )
                nc.vector.tensor_add(rec[:], r[:, 0:1], r[:, 1:2])
                nc.vector.reciprocal(rec[:], rec[:])
                nc.vector.tensor_scalar_mul(a_t[:], a_t[:], rec[:])

            def transposes(t):
                q, t2 = t // 2, t % 2
                if t2 == 0:
                    at_tiles[q] = atT_pool.tile(
                        [P, SJT, 256], bf16, tag="atT", name=f"at_q{q}"
                    )
                a_t = attn_tiles[t]
                for j8 in range(2):
                    pb = psB.tile([P, 1024], bf16, tag="psB")
                    for jj in range(8):
                        j = j8 * 8 + jj
                        nc.tensor.transpose(
                            pb[:, jj * P:(jj + 1) * P],
                            a_t[:, j * P:(j + 1) * P],
                            ident[:],
                        )
                    nc.vector.tensor_copy(
                        at_tiles[q][:, j8 * 8:(j8 + 1) * 8, t2 * P:(t2 + 1) * P],
                        pb[:].rearrange("p (j c) -> p j c", c=P),
                    )
                attn_tiles[t] = None

            def m_chains(q):
                at_sb = at_tiles[q]
                for dc in range(ET):
                    ps = psA.tile([P, 1024], f32, tag="psA")
                    for j in range(SJT):
                        nc.tensor.matmul(
                            ps[:, 0:256],
                            lhsT=xn_sb[:, j, dc * P:(dc + 1) * P],
                            rhs=at_sb[:, j, :],
                            start=(j == 0), stop=(j == SJT - 1),
                        )
                    nc.vector.tensor_copy(m_sb[:, dc, q * 256:(q + 1) * 256], ps[:, 0:256])

            scores_softmax(0)
            scores_softmax(1)
            for t in range(2, SIT):
                scores_softmax(t)
                transposes(t - 2)
                if t % 2 == 1:
                    m_chains((t - 2) // 2)
            transposes(SIT - 2)
            transposes(SIT - 1)
            m_chains(3)

            # ---- head^T = Wv^T @ M : [e, si]
            ht_sb = ht_pool.tile([P, ET, SI], bf16, tag="ht")
            for eb in range(ET):
                wv_eb = wqkv_pool.tile([P, ET, P], bf16, tag="wqkv")
                nc.sync.dma_start(out=wv_eb[:], in_=A["wvb"][h, eb])
                ps = psA.tile([P, 1024], f32, tag="psA")
                for nb in range(2):
                    for kc in range(ET):
                        nc.tensor.matmul(
                            ps[:, nb * 512:(nb + 1) * 512],
                            lhsT=wv_eb[:, kc, :],
                            rhs=m_sb[:, kc, nb * 512:(nb + 1) * 512],
                            start=(kc == 0), stop=(kc == ET - 1),
                        )
                nc.vector.tensor_copy(ht_sb[:, eb, :], ps[:])

            # ---- proj += head_h @ W1_h (+ cvec once)
            w1_tiles = []
            for eb in range(ET):
                w1_eb = w1_pool.tile([P, D], bf16, tag="w1")
                nc.sync.dma_start(
                    out=w1_eb[:], in_=A["w1"][(h * ET + eb) * P:(h * ET + eb + 1) * P, :]
                )
                w1_tiles.append(w1_eb)
            for t in range(SIT):
                ps = psA.tile([P, 1024], f32, tag="psA")
                for nb in range(2):
                    for eb in range(ET):
                        nc.tensor.matmul(
                            ps[:, nb * 512:(nb + 1) * 512],
                            lhsT=ht_sb[:, eb, t * P:(t + 1) * P],
                            rhs=w1_tiles[eb][:, nb * 512:(nb + 1) * 512],
                            start=(eb == 0), stop=(eb == ET - 1 and h != 0),
                        )
                    if h == 0:
                        nc.tensor.matmul(
                            ps[:, nb * 512:(nb + 1) * 512],
                            lhsT=ones_sb[:, :],
                            rhs=cvec_sb[:, nb * 512:(nb + 1) * 512],
                            start=False, stop=True,
                        )
                if h == 0:
                    nc.scalar.copy(proj_sb[:, t, :], ps[:])
                else:
                    nc.vector.tensor_add(proj_sb[:, t, :], proj_sb[:, t, :], ps[:])

        head_ctx.close()

        # ================= LN1 -> FFN2 -> LN2, fully per-si-tile =================
        with ExitStack() as lctx:
            lnp = lctx.enter_context(tc.tile_pool(name="lnp", bufs=1))
            xr_pool = lctx.enter_context(tc.tile_pool(name="xr", bufs=3))
            u_pool = lctx.enter_context(tc.tile_pool(name="up", bufs=3))
            sq_pool = lctx.enter_context(tc.tile_pool(name="sq", bufs=2))
            ybf_pool = lctx.enter_context(tc.tile_pool(name="ybf", bufs=2))
            yt_pool = lctx.enter_context(tc.tile_pool(name="yt", bufs=3))
            w2_pool = lctx.enter_context(tc.tile_pool(name="w2", bufs=8))
            st_pool = lctx.enter_context(tc.tile_pool(name="st", bufs=8))
            ot_pool = lctx.enter_context(tc.tile_pool(name="ot", bufs=3))

            if not trivial_gbe:
                gbe_sb = lnp.tile([P, 4, D], f32, tag="gbe")
                gbe_bc = bass.AP(
                    tensor=A["gbe"].tensor, offset=A["gbe"].offset,
                    ap=[[0, P], A["gbe"].ap[0], A["gbe"].ap[1]],
                )
                nc.gpsimd.dma_start(out=gbe_sb[:], in_=gbe_bc)
            y_sb = lnp.tile([P, SIT, D], f32, tag="y")

            xr_tiles = []
            for t in range(SIT):
                xr = xr_pool.tile([P, D], f32, tag="xr", name=f"xr{t}")
                nc.sync.dma_start(out=xr[:], in_=A["xres"][t * P:(t + 1) * P, :])
                xr_tiles.append(xr)

            w2_tiles = []
            for kc in range(ET):
                w2_kc = w2_pool.tile([P, D], bf16, tag="w2")
                nc.sync.dma_start(out=w2_kc[:], in_=A["w2"][kc * P:(kc + 1) * P, :])
                w2_tiles.append(w2_kc)

            def ln_stats(src, rsum):
                """-> (mu, rstd) [P,1] tiles from src [P,D] + its row-sum."""
                sq = sq_pool.tile([P, D], f32, tag="sq")
                sumsq = st_pool.tile([P, 1], f32, tag="sumsq")
                nc.scalar.activation(out=sq[:], in_=src, func=Act.Square,
                                     accum_out=sumsq[:])
                mu = st_pool.tile([P, 1], f32, tag="mu")
                nc.scalar.mul(mu[:], rsum, 1.0 / D)
                # (rsum*mu - sumsq) = -D*var;  std = sqrt(-1/D * that + eps)
                nv = st_pool.tile([P, 1], f32, tag="nv")
                nc.vector.scalar_tensor_tensor(
                    out=nv[:], in0=rsum, scalar=mu[:], in1=sumsq[:],
                    op0=Alu.mult, op1=Alu.subtract,
                )
                rstd = st_pool.tile([P, 1], f32, tag="rstd")
                nc.scalar.activation(out=rstd[:], in_=nv[:], func=Act.Sqrt,
                                     scale=-1.0 / D, bias=eps_sb[:])
                nc.vector.reciprocal(rstd[:], rstd[:])
                return mu, rstd

            for t in range(SIT):
                # u1 = x + proj, with row-sum accumulated in the same pass
                u1 = u_pool.tile([P, D], f32, tag="u")
                rs1 = st_pool.tile([P, 1], f32, tag="rs")
                nc.vector.scalar_tensor_tensor(
                    out=u1[:], in0=xr_tiles[t][:], scalar=1.0,
                    in1=proj_sb[:, t, :], op0=Alu.mult, op1=Alu.add,
                    accum_out=rs1[:],
                )
                mu1, rstd1 = ln_stats(u1[:], rs1[:])
                yt_t = y_sb[:, t, :]
                nc.vector.tensor_scalar(
                    yt_t, u1[:], scalar1=mu1[:], scalar2=rstd1[:],
                    op0=Alu.subtract, op1=Alu.mult,
                )
                if not trivial_gbe:
                    nc.gpsimd.tensor_mul(yt_t, yt_t, gbe_sb[:, 0, :])
                    nc.gpsimd.tensor_add(yt_t, yt_t, gbe_sb[:, 1, :])
                yb = ybf_pool.tile([P, D], bf16, tag="ybf")
                nc.scalar.copy(yb[:], yt_t)
                # transpose this tile's 8 f-blocks -> yT columns for its z-chain
                yt_tile = yt_pool.tile([P, ET, P], bf16, tag="yt")
                pb = psB.tile([P, 1024], bf16, tag="psB")
                for fb in range(ET):
                    nc.tensor.transpose(
                        pb[:, fb * P:(fb + 1) * P], yb[:, fb * P:(fb + 1) * P], ident[:]
                    )
                nc.vector.tensor_copy(
                    yt_tile[:], pb[:].rearrange("p (f c) -> p f c", c=P)
                )
                # z-chain: u2 = y + yhat @ W2' + bu
                ps = psA.tile([P, 1024], f32, tag="psA")
                for nb in range(2):
                    for kc in range(ET):
                        nc.tensor.matmul(
                            ps[:, nb * 512:(nb + 1) * 512],
                            lhsT=yt_tile[:, kc, :],
                            rhs=w2_tiles[kc][:, nb * 512:(nb + 1) * 512],
                            start=(kc == 0), stop=False,
                        )
                    nc.tensor.matmul(
                        ps[:, nb * 512:(nb + 1) * 512],
                        lhsT=ones_sb[:, :],
                        rhs=buv_sb[:, nb * 512:(nb + 1) * 512],
                        start=False, stop=True,
                    )
                u2 = u_pool.tile([P, D], f32, tag="u")
                rs2 = st_pool.tile([P, 1], f32, tag="rs")
                nc.vector.scalar_tensor_tensor(
                    out=u2[:], in0=y_sb[:, t, :], scalar=1.0,
                    in1=ps[:], op0=Alu.mult, op1=Alu.add,
                    accum_out=rs2[:],
                )
                mu2, rstd2 = ln_stats(u2[:], rs2[:])
                ot = ot_pool.tile([P, D], f32, tag="ot")
                nc.vector.tensor_scalar(
                    ot[:], u2[:], scalar1=mu2[:], scalar2=rstd2[:],
                    op0=Alu.subtract, op1=Alu.mult,
                )
                if not trivial_gbe:
                    nc.gpsimd.tensor_mul(ot[:], ot[:], gbe_sb[:, 2, :])
                    nc.gpsimd.tensor_add(ot[:], ot[:], gbe_sb[:, 3, :])
                nc.sync.dma_start(out=A["out"][t * P:(t + 1) * P, :], in_=ot[:])


def _build(trivial_gbe):
    import concourse.bass as bass
    import concourse.mybir as mybir
    import concourse.tile as tile
    from concourse import bacc

    f32 = mybir.dt.float32
    bf16 = mybir.dt.bfloat16

    nc = bacc.Bacc("TRN2", target_bir_lowering=False, debug=False, num_devices=8)
    A = {}

    def din(name, shape, dt):
        A[name] = nc.dram_tensor(name, shape, dt, kind="ExternalInput").ap()

    din("xt", [D, S], bf16)
    din("xn", [S, D], bf16)
    din("xres", [SI, D], f32)
    din("wqb", [H, ET, P, ET, P], bf16)
    din("wkb", [H, ET, P, ET, P], bf16)
    din("wvb", [H, ET, P, ET, P], bf16)
    din("w1", [H * D, D], bf16)
    din("w2", [D, D], bf16)
    din("bqr", [P, H * ET], f32)
    din("cvec", [1, D], bf16)
    din("buv", [1, D], bf16)
    if not trivial_gbe:
        din("gbe", [4, D], f32)
    A["out"] = nc.dram_tensor("out", [SI, D], f32, kind="ExternalOutput").ap()

    with tile.TileContext(nc) as tc:
        _emit(nc, tc, A, trivial_gbe)
    nc.compile()
    return nc


def _get_nc(trivial_gbe=True):
    key = ("nc", trivial_gbe)
    if key not in _CACHE:
        _CACHE[key] = _build(trivial_gbe)
    return _CACHE[key]


def _prep_inputs(inputs):
    x = np.ascontiguousarray(inputs["embedding_matrix"], dtype=np.float32)
    Wq = np.asarray(inputs["Wq"], np.float32)
    bq = np.asarray(inputs["bq"], np.float32)
    Wv = np.asarray(inputs["Wv"], np.float32)
    bv = np.asarray(inputs["bv"], np.float32)
    Wk = np.asarray(inputs["Wk"], np.float32)
    W1 = np.asarray(inputs["W1"], np.float32)
    b1 = np.asarray(inputs["b1"], np.float32)
    W2 = np.asarray(inputs["W2"], np.float32)
    b2 = np.asarray(inputs["b2"], np.float32)
    g1 = np.asarray(inputs["g1"], np.float32)
    be1 = np.asarray(inputs["be1"], np.float32)
    g2 = np.asarray(inputs["g2"], np.float32)
    be2 = np.asarray(inputs["be2"], np.float32)

    trivial = (
        np.array_equal(g1, np.ones(D, np.float32))
        and np.array_equal(g2, np.ones(D, np.float32))
        and np.array_equal(be1, np.zeros(D, np.float32))
        and np.array_equal(be2, np.zeros(D, np.float32))
    )

    def pack_w(W):  # [H, D, D] -> [H, ET, P(row-in-block), ET(kc), P] lhsT blocks
        return np.ascontiguousarray(
            W.reshape(H, ET, P, ET, P).transpose(0, 3, 2, 1, 4).astype(BF)
        )

    wqb = pack_w(Wq)
    wkb = pack_w(Wk)
    wvb = pack_w(Wv)
    w1b = np.ascontiguousarray(W1.astype(BF))
    w2b = np.ascontiguousarray(W2.astype(BF))
    buv = b2.astype(np.float32)
    # bq rearranged so bias for (h, e-block c) is column h*ET+c: [P, H*ET]
    bqr = np.ascontiguousarray(bq.reshape(H, ET, P).transpose(2, 0, 1).reshape(P, H * ET))
    cvec = (b1 + sum(bv[h] @ W1[h * D:(h + 1) * D] for h in range(H)))
    cvec = np.ascontiguousarray(cvec.reshape(1, D).astype(BF))
    buv = np.ascontiguousarray(buv.reshape(1, D).astype(BF))

    shared = {
        "wqb": wqb, "wkb": wkb, "wvb": wvb, "w1": w1b, "w2": w2b,
        "bqr": bqr, "cvec": cvec, "buv": buv,
    }
    if not trivial:
        shared["gbe"] = np.ascontiguousarray(np.stack([g1, be1, g2, be2]))
    in_maps = []
    for core in range(8):
        b, half = core // 2, core % 2
        own = x[b, half * SI:(half + 1) * SI]
        other = x[b, (1 - half) * SI:(2 - half) * SI]
        xperm = np.concatenate([own, other], axis=0)
        m = dict(shared)
        m["xn"] = np.ascontiguousarray(xperm.astype(BF))
        m["xt"] = np.ascontiguousarray(xperm.T.astype(BF))
        m["xres"] = np.ascontiguousarray(own)
        in_maps.append(m)
    return trivial, in_maps


def kernel(**inputs):
    from concourse.bass_utils import run_bass_kernel_spmd

    trivial, in_maps = _prep_inputs(inputs)
    nc = _get_nc(trivial)
    res = run_bass_kernel_spmd(nc, in_maps, core_ids=list(range(8)))
    out = np.empty((4, S, D), np.float32)
    for core in range(8):
        b, half = core // 2, core % 2
        out[b, half * SI:(half + 1) * SI] = res.results[core]["out"]
    return out



# revision 2
# speedup vs baseline: 1.0162x; 1.0162x over previous
"""Trainium2 Bass kernel for nn_EncoderOnlyBlock (4-head full-dim encoder block).

Sharding: data-parallel, 8 cores = (batch b, seq-half), with a pairwise
K^T AllGather so each core projects only its own 1024 tokens through Wk
(the only cross-core duplicated work in v2/v3).

v4 = v3 (fp8 DoubleRow attention, transposed scores with deferred softmax
normalization, Phase B interleaved with head 3) plus:
  - K-exchange: per head, each core computes K^T for its own half, DMAs it
    to a DRAM bounce buffer, AllGathers across the core pair, and reads the
    full [e, 2048] K^T back. kt/at/xn all use the gathered (natural) token
    order, which is core-independent, so one SPMD program serves all cores.
    K-own for head h+1 is computed mid-head-h so the collective latency
    hides behind the M/ht/proj chains.
  - Phase B z-chain lookahead of 2 proj chains (the LN1 critical path is
    longer than one chain), LN2 output scaling on gpsimd, LN stats with
    fewer cross-engine hops.

See kernel_v2/v3 docstrings for the math derivation.
"""

import numpy as np
import ml_dtypes

BF = ml_dtypes.bfloat16
F8 = ml_dtypes.float8_e4m3
P = 128
D = 1024
S = 2048
SI = 1024
H = 4
ET = D // P       # 8 e/d/f 128-blocks
SJT = S // P      # 16 sj 128-blocks
SIT = SI // P     # 8 si 128-blocks
SCALE = 1.0 / 32.0  # 1/sqrt(D)
LN4 = float(np.log(4.0))
EPS = 1e-5

_CACHE = {}


def _emit(nc, tc, A, trivial_gbe):
    """Emit the per-core program. A: dict name -> dram AP."""
    from contextlib import ExitStack

    import concourse.bass as bass
    import concourse.mybir as mybir
    from concourse.masks import make_identity

    f32 = mybir.dt.float32
    bf16 = mybir.dt.bfloat16
    fp8 = mybir.dt.float8e4
    Act = mybir.ActivationFunctionType
    Alu = mybir.AluOpType
    DR = mybir.MatmulPerfMode.DoubleRow
    PAIRS = [[0, 1], [2, 3], [4, 5], [6, 7]]

    with ExitStack() as ctx:
        consts = ctx.enter_context(tc.tile_pool(name="consts", bufs=1))
        psA = ctx.enter_context(tc.tile_pool(name="psA", bufs=3, space="PSUM"))
        psB = ctx.enter_context(tc.tile_pool(name="psB", bufs=2, space="PSUM"))
        dram = ctx.enter_context(tc.tile_pool(name="dram", bufs=4, space="DRAM"))

        ident = consts.tile([P, P], bf16, tag="ident")
        make_identity(nc, ident[:])
        bqr_sb = consts.tile([P, H * ET], f32, tag="bqr")
        nc.sync.dma_start(out=bqr_sb[:], in_=A["bqr"][:])
        buv_sb = consts.tile([1, D], bf16, tag="buv")
        nc.sync.dma_start(out=buv_sb[:], in_=A["buv"][:])
        ones8_sb = consts.tile([P, 2, 16], fp8, tag="ones8")
        nc.sync.dma_start(out=ones8_sb[:], in_=A["ones8"][:])
        ones_sb = consts.tile([1, P], bf16, tag="ones")
        nc.vector.memset(ones_sb[:], 1.0)
        eps_sb = consts.tile([P, 1], f32, tag="eps")
        nc.vector.memset(eps_sb[:], EPS)
        nln4_sb = consts.tile([P, 1], f32, tag="nln4")
        nc.vector.memset(nln4_sb[:], -LN4)
        # per-head, per-si 1/colsum scalars and their staging rows
        rec_sb = consts.tile([P, H, SIT], f32, tag="rec")
        csT_sb = consts.tile([P, H, SIT], f32, tag="csT")
        cs_pool = ctx.enter_context(tc.tile_pool(name="cs", bufs=1))

        xpool = ctx.enter_context(tc.tile_pool(name="xp", bufs=1))
        proj_pool = ctx.enter_context(tc.tile_pool(name="pj", bufs=1))
        wqkv_pool = ctx.enter_context(tc.tile_pool(name="wqkv", bufs=3))
        w1_pool = ctx.enter_context(tc.tile_pool(name="w1", bufs=2))
        kt_pool = ctx.enter_context(tc.tile_pool(name="kt", bufs=2))
        kto_pool = ctx.enter_context(tc.tile_pool(name="kto", bufs=2))

        # x^T (own half only; first K-own chain consumes it), x natural later
        xt_sb = xpool.tile([P, ET, SI], fp8, tag="xt")
        for c in range(ET):
            eng = (nc.sync, nc.gpsimd, nc.scalar)[c % 3]
            eng.dma_start(out=xt_sb[:, c, :], in_=A["xt"][c * P:(c + 1) * P, :])
        xn_sb = xpool.tile([P, SJT, D], fp8, tag="xn")

        proj_sb = proj_pool.tile([P, SIT, D], bf16, tag="proj")

        kt_tiles = [None] * H

        def k_exchange(h):
            """Compute K^T for own tokens, AllGather across the pair, read the
            full natural-order K^T back into kt_tiles[h]."""
            kx = dram.tile([D, SI], fp8, tag="dram", name=f"kx{h}")
            kg = dram.tile([2, D, SI], fp8, tag="dram", name=f"kg{h}")
            for c in range(ET):
                wk_c = wqkv_pool.tile([P, ET, P], fp8, tag="wqkv", name=f"wk{h}_{c}")
                nc.sync.dma_start(out=wk_c[:], in_=A["wkb"][h, c])
                ps = psA.tile([P, 1024], f32, tag="psA")
                for nb in range(2):
                    for kp in range(4):
                        nc.tensor.matmul(
                            ps[:, nb * 512:(nb + 1) * 512],
                            lhsT=wk_c[:, 2 * kp:2 * kp + 2, :],
                            rhs=xt_sb[:, 2 * kp:2 * kp + 2, nb * 512:(nb + 1) * 512],
                            start=(kp == 0), stop=(kp == 3),
                            perf_mode=DR,
                        )
                kto = kto_pool.tile([P, SI], fp8, tag="kto", name=f"kto{h}_{c}")
                nc.vector.tensor_copy(kto[:], ps[:])
                nc.scalar.dma_start(out=kx[c * P:(c + 1) * P, :], in_=kto[:])
            nc.gpsimd.collective_compute(
                "AllGather", mybir.AluOpType.bypass,
                replica_groups=PAIRS,
                ins=[kx.opt()], outs=[kg.opt()],
            )
            kt_sb = kt_pool.tile([P, ET, S], fp8, tag="kt", name=f"kt{h}")
            for g in range(2):
                nc.gpsimd.dma_start(
                    out=kt_sb[:, :, g * SI:(g + 1) * SI],
                    in_=kg[g].rearrange("(c p) s -> p c s", p=P),
                )
            kt_tiles[h] = kt_sb

        qt_tiles = [None] * H

        def emit_q(h, hp):
            # ---- Q^T = Wq^T @ x^T + bq : [e, si]
            qt_sb = hp["qt"].tile([P, ET, SI], fp8, tag="qt", name=f"qt{h}")
            qt_tiles[h] = qt_sb
            for c in range(ET):
                wq_c = wqkv_pool.tile([P, ET, P], fp8, tag="wqkv", name=f"wq{h}_{c}")
                nc.sync.dma_start(out=wq_c[:], in_=A["wqb"][h, c])
                ps = psA.tile([P, 1024], f32, tag="psA")
                for nb in range(2):
                    for kp in range(4):
                        nc.tensor.matmul(
                            ps[:, nb * 512:(nb + 1) * 512],
                            lhsT=wq_c[:, 2 * kp:2 * kp + 2, :],
                            rhs=xt_sb[:, 2 * kp:2 * kp + 2, nb * 512:(nb + 1) * 512],
                            start=(kp == 0), stop=(kp == 3),
                            perf_mode=DR,
                        )
                nc.scalar.activation(
                    out=qt_sb[:, c, :], in_=ps[:], func=Act.Identity,
                    bias=bqr_sb[:, h * ET + c:h * ET + c + 1],
                )

        def emit_head(h, hp, phase_b, skip_q=False):
            """One attention head. hp: dict of per-head pools.
            phase_b: callback(t) emitted after proj chain t+2 (None for h<3)."""
            if not skip_q:
                emit_q(h, hp)
            qt_sb = qt_tiles[h]

            if h == 0:
                for j in range(SJT):
                    nc.sync.dma_start(out=xn_sb[:, j, :], in_=A["xn"][j * P:(j + 1) * P, :])
                w1_sb = w1_pool.tile([P, ET, D], fp8, tag="w1", name="w1_0")
                nc.sync.dma_start(out=w1_sb[:], in_=A["w1"][0])
            else:
                w1_sb = w1_tiles[h]

            # ---- A^T = exp(S^T/sqrt(D) - ln4) per sj-block : [sj, si]
            kt_sb = kt_tiles[h]
            at_sb = hp["at"].tile([P, SJT, SI], fp8, tag="at")
            for j in range(SJT):
                ps = psA.tile([P, 1024], f32, tag="psA")
                for nb in range(2):
                    for kp in range(4):
                        nc.tensor.matmul(
                            ps[:, nb * 512:(nb + 1) * 512],
                            lhsT=kt_sb[:, 2 * kp:2 * kp + 2, j * P:(j + 1) * P],
                            rhs=qt_sb[:, 2 * kp:2 * kp + 2, nb * 512:(nb + 1) * 512],
                            start=(kp == 0), stop=(kp == 3),
                            perf_mode=DR,
                        )
                nc.scalar.activation(
                    out=at_sb[:, j, :], in_=ps[:], func=Act.Exp,
                    scale=SCALE, bias=nln4_sb[:],
                )

            # ---- colsum(A^T) = softmax rowsums (carrying the same 1/4)
            cs_sb = cs_pool.tile([1, SI], f32, tag="cs", name=f"cs{h}")
            for nb in range(2):
                cs_ps = psB.tile([16, 512], f32, tag="psB", name=f"cs{h}_{nb}")
                for jp in range(8):
                    nc.tensor.matmul(
                        cs_ps[:],
                        lhsT=ones8_sb[:],
                        rhs=at_sb[:, 2 * jp:2 * jp + 2, nb * 512:(nb + 1) * 512],
                        start=(jp == 0), stop=(jp == 7),
                        perf_mode=DR,
                    )
                nc.vector.tensor_copy(cs_sb[:, nb * 512:(nb + 1) * 512],
                                      cs_ps[0:1, :])
            # [1, SI] row -> [P, SIT] partition layout via a DRAM round-trip on
            # one FIFO DMA queue (write row, gather back transposed), then 1/x
            nc.sync.dma_start(out=A["csr"][h:h + 1, :], in_=cs_sb[:])
            csr_t = bass.AP(
                tensor=A["csr"].tensor, offset=A["csr"].offset + h * SI,
                ap=[[1, P], [P, SIT]],
            )
            nc.sync.dma_start(out=csT_sb[:, h, :], in_=csr_t)
            nc.vector.reciprocal(rec_sb[:, h, :], csT_sb[:, h, :])

            # ---- K-own + exchange (first three are in the prologue)
            if h == 1:
                k_exchange(3)
            if h < H - 1:
                w1n = w1_pool.tile([P, ET, D], fp8, tag="w1", name=f"w1_{h+1}")
                nc.sync.dma_start(out=w1n[:], in_=A["w1"][h + 1])
                w1_tiles[h + 1] = w1n

            # ---- M = x^T @ A^T : [d, si]
            m_sb = hp["m"].tile([P, ET, SI], fp8, tag="m")
            for dc in range(ET):
                ps = psA.tile([P, 1024], f32, tag="psA")
                for nb in range(2):
                    for jp in range(8):
                        nc.tensor.matmul(
                            ps[:, nb * 512:(nb + 1) * 512],
                            lhsT=xn_sb[:, 2 * jp:2 * jp + 2, dc * P:(dc + 1) * P],
                            rhs=at_sb[:, 2 * jp:2 * jp + 2, nb * 512:(nb + 1) * 512],
                            start=(jp == 0), stop=(jp == 7),
                            perf_mode=DR,
                        )
                nc.vector.tensor_copy(m_sb[:, dc, :], ps[:])

            # ---- head^T = Wv^T @ M : [e, si]
            ht_sb = hp["ht"].tile([P, ET, SI], fp8, tag="ht")
            for eb in range(ET):
                wv_eb = wqkv_pool.tile([P, ET, P], fp8, tag="wqkv", name=f"wv{h}_{eb}")
                nc.sync.dma_start(out=wv_eb[:], in_=A["wvb"][h, eb])
                ps = psA.tile([P, 1024], f32, tag="psA")
                for nb in range(2):
                    for kp in range(4):
                        nc.tensor.matmul(
                            ps[:, nb * 512:(nb + 1) * 512],
                            lhsT=wv_eb[:, 2 * kp:2 * kp + 2, :],
                            rhs=m_sb[:, 2 * kp:2 * kp + 2, nb * 512:(nb + 1) * 512],
                            start=(kp == 0), stop=(kp == 3),
                            perf_mode=DR,
                        )
                nc.scalar.copy(ht_sb[:, eb, :], ps[:])

            # ---- proj += r_h * (head_h @ W1_h)
            for t in range(SIT):
                ps = psA.tile([P, 1024], f32, tag="psA")
                for nb in range(2):
                    for ep in range(4):
                        nc.tensor.matmul(
                            ps[:, nb * 512:(nb + 1) * 512],
                            lhsT=ht_sb[:, 2 * ep:2 * ep + 2, t * P:(t + 1) * P],
                            rhs=w1_sb[:, 2 * ep:2 * ep + 2, nb * 512:(nb + 1) * 512],
                            start=(ep == 0), stop=(ep == 3),
                            perf_mode=DR,
                        )
                if h == 0:
                    nc.vector.tensor_scalar_mul(
                        proj_sb[:, t, :], ps[:], rec_sb[:, 0, t:t + 1],
                    )
                else:
                    nc.vector.scalar_tensor_tensor(
                        out=proj_sb[:, t, :], in0=ps[:],
                        scalar=rec_sb[:, h, t:t + 1],
                        in1=proj_sb[:, t, :], op0=Alu.mult, op1=Alu.add,
                    )
                if phase_b is not None and t >= 2:
                    phase_b(t - 2)
            if phase_b is not None:
                phase_b(SIT - 2)
                phase_b(SIT - 1)
                phase_b(SIT)
                phase_b(SIT + 1)

        w1_tiles = [None] * H

        # -------- prologue: local K-full for head 0 (no collective on the
        # critical path), exchanges for heads 1+2, Q for head 0 --------
        head_ctx = ExitStack()
        hp = {n: head_ctx.enter_context(tc.tile_pool(name=n, bufs=1))
              for n in ("qt", "at", "m", "ht")}
        xtf_pool = head_ctx.enter_context(tc.tile_pool(name="xtf", bufs=1))
        xtf_sb = xtf_pool.tile([P, ET, S], fp8, tag="xtf")
        for c in range(ET):
            eng = (nc.sync, nc.gpsimd, nc.scalar)[c % 3]
            eng.dma_start(out=xtf_sb[:, c, :], in_=A["xtf"][c * P:(c + 1) * P, :])
        kt0_sb = kt_pool.tile([P, ET, S], fp8, tag="kt", name="kt0")
        for c in range(ET):
            wk_c = wqkv_pool.tile([P, ET, P], fp8, tag="wqkv", name=f"wk0_{c}")
            nc.sync.dma_start(out=wk_c[:], in_=A["wkb"][0, c])
            for hs in range(2):
                ps = psA.tile([P, 1024], f32, tag="psA")
                for nb in range(2):
                    for kp in range(4):
                        nc.tensor.matmul(
                            ps[:, nb * 512:(nb + 1) * 512],
                            lhsT=wk_c[:, 2 * kp:2 * kp + 2, :],
                            rhs=xtf_sb[:, 2 * kp:2 * kp + 2,
                                       hs * 1024 + nb * 512:hs * 1024 + (nb + 1) * 512],
                            start=(kp == 0), stop=(kp == 3),
                            perf_mode=DR,
                        )
                nc.vector.tensor_copy(kt0_sb[:, c, hs * 1024:(hs + 1) * 1024], ps[:])
        kt_tiles[0] = kt0_sb
        k_exchange(1)
        k_exchange(2)
        emit_q(0, hp)

        # ---------------- heads 0..2 ----------------
        for h in range(H - 1):
            emit_head(h, hp, None, skip_q=(h < 1))
        head_ctx.close()

        # ---------------- head 3 + Phase B interleaved ----------------
        with ExitStack() as lctx:
            hp = {n: lctx.enter_context(tc.tile_pool(name=n + "3", bufs=1))
                  for n in ("qt", "at", "m", "ht")}
            lnp = lctx.enter_context(tc.tile_pool(name="lnp", bufs=1))
            xr_pool = lctx.enter_context(tc.tile_pool(name="xr", bufs=2))
            u_pool = lctx.enter_context(tc.tile_pool(name="up", bufs=4))
            sq_pool = lctx.enter_context(tc.tile_pool(name="sq", bufs=1))
            yt_pool = lctx.enter_context(tc.tile_pool(name="yt", bufs=2))
            w2_pool = lctx.enter_context(tc.tile_pool(name="w2", bufs=8))
            st_pool = lctx.enter_context(tc.tile_pool(name="st", bufs=8))
            ot_pool = lctx.enter_context(tc.tile_pool(name="ot", bufs=1))

            if not trivial_gbe:
                gbe_sb = lnp.tile([P, 4, D], f32, tag="gbe")
                gbe_bc = bass.AP(
                    tensor=A["gbe"].tensor, offset=A["gbe"].offset,
                    ap=[[0, P], A["gbe"].ap[0], A["gbe"].ap[1]],
                )
                nc.gpsimd.dma_start(out=gbe_sb[:], in_=gbe_bc)
            y_sb = lnp.tile([P, SIT, D], bf16, tag="y")

            w2_tiles = []
            for kc in range(ET):
                w2_kc = w2_pool.tile([P, D], bf16, tag="w2")
                nc.scalar.dma_start(out=w2_kc[:], in_=A["w2"][kc * P:(kc + 1) * P, :])
                w2_tiles.append(w2_kc)

            xr_tiles = [None] * SIT

            def fetch_xr(t):
                if t < SIT:
                    xr = xr_pool.tile([P, D], f32, tag="xr", name=f"xr{t}")
                    nc.scalar.dma_start(out=xr[:], in_=A["xres"][t * P:(t + 1) * P, :])
                    xr_tiles[t] = xr

            def ln_stats(src, rsum):
                """-> (mu, rstd) [P,1] tiles from src [P,D] + its row-sum.
                sq runs on scalar; everything else stays on DVE to minimize
                cross-engine hops on the critical path."""
                sq = sq_pool.tile([P, D], bf16, tag="sq")
                sumsq = st_pool.tile([P, 1], f32, tag="sumsq")
                nc.scalar.activation(out=sq[:], in_=src, func=Act.Square,
                                     accum_out=sumsq[:])
                mu = st_pool.tile([P, 1], f32, tag="mu")
                nc.vector.tensor_scalar_mul(mu[:], rsum, 1.0 / D)
                # (rsum*mu - sumsq) = -D*var;  std = sqrt(-1/D * that + eps)
                nv = st_pool.tile([P, 1], f32, tag="nv")
                nc.vector.scalar_tensor_tensor(
                    out=nv[:], in0=rsum, scalar=mu[:], in1=sumsq[:],
                    op0=Alu.mult, op1=Alu.subtract,
                )
                rstd = st_pool.tile([P, 1], f32, tag="rstd")
                nc.scalar.activation(out=rstd[:], in_=nv[:], func=Act.Sqrt,
                                     scale=-1.0 / D, bias=eps_sb[:])
                nc.vector.reciprocal(rstd[:], rstd[:])
                return mu, rstd

            b2_state = {}

            def phase_b(t):
                if t < SIT:
                    phase_b1(t)
                if t >= 2 and t - 2 < SIT:
                    phase_b2(t - 2)

            def phase_b1(t):
                # u1 = (x + cvec) + proj, with row-sum accumulated in the same pass
                u1 = u_pool.tile([P, D], f32, tag="u", name=f"u1_{t}")
                rs1 = st_pool.tile([P, 1], f32, tag="rs")
                nc.vector.scalar_tensor_tensor(
                    out=u1[:], in0=xr_tiles[t][:], scalar=1.0,
                    in1=proj_sb[:, t, :], op0=Alu.mult, op1=Alu.add,
                    accum_out=rs1[:],
                )
                fetch_xr(t + 2)
                mu1, rstd1 = ln_stats(u1[:], rs1[:])
                yt_t = y_sb[:, t, :]
                nc.vector.tensor_scalar(
                    yt_t, u1[:], scalar1=mu1[:], scalar2=rstd1[:],
                    op0=Alu.subtract, op1=Alu.mult,
                )
                if not trivial_gbe:
                    nc.gpsimd.tensor_mul(yt_t, yt_t, gbe_sb[:, 0, :])
                    nc.gpsimd.tensor_add(yt_t, yt_t, gbe_sb[:, 1, :])
                # transpose this tile's 8 f-blocks -> yT columns for its z-chain
                yt_tile = yt_pool.tile([P, ET, P], bf16, tag="yt")
                pb = psB.tile([P, 1024], bf16, tag="psB")
                for fb in range(ET):
                    nc.tensor.transpose(
                        pb[:, fb * P:(fb + 1) * P], yt_t[:, fb * P:(fb + 1) * P],
                        ident[:],
                    )
                nc.vector.tensor_copy(
                    yt_tile[:], pb[:].rearrange("p (f c) -> p f c", c=P)
                )
                # z-chain: u2 = y + yhat @ W2' + bu
                ps = psA.tile([P, 1024], f32, tag="psA")
                for nb in range(2):
                    for kc in range(ET):
                        nc.tensor.matmul(
                            ps[:, nb * 512:(nb + 1) * 512],
                            lhsT=yt_tile[:, kc, :],
                            rhs=w2_tiles[kc][:, nb * 512:(nb + 1) * 512],
                            start=(kc == 0), stop=False,
                        )
                    nc.tensor.matmul(
                        ps[:, nb * 512:(nb + 1) * 512],
                        lhsT=ones_sb[:, :],
                        rhs=buv_sb[:, nb * 512:(nb + 1) * 512],
                        start=False, stop=True,
                    )
                b2_state[t] = ps

            def phase_b2(t):
                ps = b2_state.pop(t)
                u2 = u_pool.tile([P, D], f32, tag="u", name=f"u2_{t}")
                rs2 = st_pool.tile([P, 1], f32, tag="rs")
                nc.vector.scalar_tensor_tensor(
                    out=u2[:], in0=y_sb[:, t, :], scalar=1.0,
                    in1=ps[:], op0=Alu.mult, op1=Alu.add,
                    accum_out=rs2[:],
                )
                mu2, rstd2 = ln_stats(u2[:], rs2[:])
                ot = ot_pool.tile([P, D], f32, tag="ot")
                nc.vector.tensor_scalar(
                    ot[:], u2[:], scalar1=mu2[:], scalar2=rstd2[:],
                    op0=Alu.subtract, op1=Alu.mult,
                )
                if not trivial_gbe:
                    nc.gpsimd.tensor_mul(ot[:], ot[:], gbe_sb[:, 2, :])
                    nc.gpsimd.tensor_add(ot[:], ot[:], gbe_sb[:, 3, :])
                nc.sync.dma_start(out=A["out"][t * P:(t + 1) * P, :], in_=ot[:])

            for t in range(2):
                fetch_xr(t)
            emit_head(H - 1, hp, phase_b)


def _build(trivial_gbe):
    import concourse.bass as bass
    import concourse.mybir as mybir
    import concourse.tile as tile
    from concourse import bacc

    f32 = mybir.dt.float32
    bf16 = mybir.dt.bfloat16
    fp8 = mybir.dt.float8e4

    nc = bacc.Bacc("TRN2", target_bir_lowering=False, debug=False, num_devices=8)
    A = {}

    def din(name, shape, dt):
        A[name] = nc.dram_tensor(name, shape, dt, kind="ExternalInput").ap()

    din("xt", [D, SI], fp8)
    din("xtf", [D, S], fp8)
    din("xn", [S, D], fp8)
    din("xres", [SI, D], f32)
    din("wqb", [H, ET, P, ET, P], fp8)
    din("wkb", [H, ET, P, ET, P], fp8)
    din("wvb", [H, ET, P, ET, P], fp8)
    din("w1", [H, P, ET, D], fp8)
    din("w2", [D, D], bf16)
    din("bqr", [P, H * ET], f32)
    din("buv", [1, D], bf16)
    din("ones8", [P, 2, 16], fp8)
    A["csr"] = nc.dram_tensor("csr", [H, SI], f32, kind="Internal").ap()
    if not trivial_gbe:
        din("gbe", [4, D], f32)
    A["out"] = nc.dram_tensor("out", [SI, D], f32, kind="ExternalOutput").ap()

    with tile.TileContext(nc) as tc:
        _emit(nc, tc, A, trivial_gbe)
    nc.compile()
    return nc


def _get_nc(trivial_gbe=True):
    key = ("nc", trivial_gbe)
    if key not in _CACHE:
        _CACHE[key] = _build(trivial_gbe)
    return _CACHE[key]


def _prep_inputs(inputs):
    x = np.ascontiguousarray(inputs["embedding_matrix"], dtype=np.float32)
    Wq = np.asarray(inputs["Wq"], np.float32)
    bq = np.asarray(inputs["bq"], np.float32)
    Wv = np.asarray(inputs["Wv"], np.float32)
    bv = np.asarray(inputs["bv"], np.float32)
    Wk = np.asarray(inputs["Wk"], np.float32)
    W1 = np.asarray(inputs["W1"], np.float32)
    b1 = np.asarray(inputs["b1"], np.float32)
    W2 = np.asarray(inputs["W2"], np.float32)
    b2 = np.asarray(inputs["b2"], np.float32)
    g1 = np.asarray(inputs["g1"], np.float32)
    be1 = np.asarray(inputs["be1"], np.float32)
    g2 = np.asarray(inputs["g2"], np.float32)
    be2 = np.asarray(inputs["be2"], np.float32)

    trivial = (
        np.array_equal(g1, np.ones(D, np.float32))
        and np.array_equal(g2, np.ones(D, np.float32))
        and np.array_equal(be1, np.zeros(D, np.float32))
        and np.array_equal(be2, np.zeros(D, np.float32))
    )

    def pack_w(W):  # [H, D, D] -> [H, ET(e-blk), P(d-in), ET(kc), P(e-in)] lhsT
        return np.ascontiguousarray(
            W.reshape(H, ET, P, ET, P).transpose(0, 3, 2, 1, 4).astype(F8)
        )

    wqb = pack_w(Wq)
    wkb = pack_w(Wk)
    wvb = pack_w(Wv)
    # [H*D, D] -> [H, P(e-in), ET(e-blk), D(f)]
    w1b = np.ascontiguousarray(
        W1.reshape(H, ET, P, D).transpose(0, 2, 1, 3).astype(F8)
    )
    w2b = np.ascontiguousarray(W2.astype(BF))
    # bq rearranged so bias for (h, e-block c) is column h*ET+c: [P, H*ET]
    bqr = np.ascontiguousarray(bq.reshape(H, ET, P).transpose(2, 0, 1).reshape(P, H * ET))
    cvec = (b1 + sum(bv[h] @ W1[h * D:(h + 1) * D] for h in range(H)))
    buv = np.ascontiguousarray(b2.reshape(1, D).astype(BF))
    ones8 = np.ones((P, 2, 16), F8)

    shared = {
        "wqb": wqb, "wkb": wkb, "wvb": wvb, "w1": w1b, "w2": w2b,
        "bqr": bqr, "buv": buv, "ones8": ones8,
    }
    if not trivial:
        shared["gbe"] = np.ascontiguousarray(np.stack([g1, be1, g2, be2]))
    in_maps = []
    for core in range(8):
        b, half = core // 2, core % 2
        own = x[b, half * SI:(half + 1) * SI]
        m = dict(shared)
        m["xn"] = np.ascontiguousarray(x[b].astype(F8))   # natural order
        m["xt"] = np.ascontiguousarray(own.T.astype(F8))  # own half only
        m["xtf"] = np.ascontiguousarray(x[b].T.astype(F8))  # natural full
        m["xres"] = np.ascontiguousarray(own + cvec[None, :])
        in_maps.append(m)
    return trivial, in_maps


def kernel(**inputs):
    from concourse.bass_utils import run_bass_kernel_spmd

    trivial, in_maps = _prep_inputs(inputs)
    nc = _get_nc(trivial)
    res = run_bass_kernel_spmd(nc, in_maps, core_ids=list(range(8)))
    out = np.empty((4, S, D), np.float32)
    for core in range(8):
        b, half = core // 2, core % 2
        out[b, half * SI:(half + 1) * SI] = res.results[core]["out"]
    return out
